# revision 13
# baseline (speedup 1.0000x reference)
"""DTNN message-passing layer on 8 Trainium2 NeuronCores.

Math (per node n, neighbor slot k):
    h_w  = src_h @ Wcf_w.T + Wcf_b          # [N,K,128]
    e_vw = he    @ Wdf_w.T + Wdf_b          # [N,K,128]
    y    = sum_k tanh((h_w * e_vw) @ Wfc_w.T + Wfc_b)   # [N,128]

Strategy: data-parallel over nodes (6250/core, padded to 6272 = 49*128).
The bond branch e_vw is tiny compute (K=32 contraction) but forcing it
through the PE costs a PSUM bank plus a PSUM->SBUF copy pass (DVE ops may
read at most one PSUM operand), so e_vw is precomputed on the host with BLAS
and shipped as bf16 -- total DMA stays ~106MB/core, well under the engine
budget. Host packs feature-major, k-major bf16 layouts:
    xt [128, K*Np]  xt[a, k*Np + n] = src_h[n, k, a]
    ev [128, K*Np]  ev[h, k*Np + n] = e_vw[n, k, h]
All matmul operands are bf16 (1 cyc/row; fast FWL weight loads -- the
4-byte LDW path costs ~233ns/load and f32r matmuls measured ~3 cyc/row).

Device, per 512-node chunk, per k-pair (free dim 1024 amortizes DVE/ACT
instruction overhead; PSUM matmul writes stay 512 wide = one bank):
    p1 = W1.T @ xt               (PE)
    hp = (p1 + bcf) * ev         (DVE scalar_tensor_tensor -> bf16 SBUF)
    p3 = W3.T @ hp               (PE)
    t  = tanh(p3 + bfc)          (ACT, per-partition bias -> bf16)
    psum_y[:, 128j:] += t-slice.T @ I   (PE: REGULAR matmul with identity
        moving operand = transpose + fp32 PSUM accumulation over all 32 k,
        i.e. the neighbor sum and the [feat,node]->[node,feat] output
        transpose are both done by the tensor engine for ~60ns/slice)
Then one PSUM->SBUF copy and one DMA per chunk writes y rows.

The walrus in this container accepts at most ONE embedded sync wait per
instruction; _split_multi_waits() moves Tile's extra waits onto same-engine
NoOps after scheduling.
"""

import sys

if "/opt/trn_rl_repo" not in sys.path:
    sys.path.insert(0, "/opt/trn_rl_repo")

import numpy as np

N_NODES = 50000
MAX_DEG = 32
ATOM = 128
BOND = 32
HID = 128
N_CORES = 8
N_PER = N_NODES // N_CORES  # 6250
N_PAD = 6272                # 49*128
CW = 512                    # nodes per chunk (psum_y = one PSUM bank)

_prog_cache = {}


def build_program(n_pad=N_PAD):
    """Build the (SPMD, per-core) bass program. Cached per shape."""
    if n_pad in _prog_cache:
        return _prog_cache[n_pad]

    from contextlib import ExitStack

    import concourse.bass as bass
    import concourse.mybir as mybir
    import concourse.tile as tile

    f32 = mybir.dt.float32
    f32r = mybir.dt.float32r
    bf16 = mybir.dt.bfloat16
    m_pad = n_pad * MAX_DEG
    add = mybir.AluOpType.add
    mult = mybir.AluOpType.mult
    tanh = mybir.ActivationFunctionType.Tanh
    fcopy = mybir.ActivationFunctionType.Copy

    nc = bass.Bass("TRN2", target_bir_lowering=False, debug=False)
    XT = nc.dram_tensor("xt", [ATOM, m_pad], bf16, kind="ExternalInput").ap()
    EV = nc.dram_tensor("ev", [HID, m_pad], bf16, kind="ExternalInput").ap()
    W1 = nc.dram_tensor("w1", [ATOM, HID], bf16, kind="ExternalInput").ap()
    W3 = nc.dram_tensor("w3", [HID, ATOM], bf16, kind="ExternalInput").ap()
    BCF = nc.dram_tensor("bcf", [HID, 1], f32, kind="ExternalInput").ap()
    BFC = nc.dram_tensor("bfc", [ATOM, 1], f32, kind="ExternalInput").ap()
    IDN = nc.dram_tensor("idn", [128, 128], bf16, kind="ExternalInput").ap()
    Y = nc.dram_tensor("y", [n_pad, ATOM], f32, kind="ExternalOutput").ap()

    XT3 = XT.rearrange("a (k n) -> a k n", n=n_pad)
    EV3 = EV.rearrange("h (k n) -> h k n", n=n_pad)

    # node chunks
    chunks = []
    c0 = 0
    while c0 < n_pad:
        cw = min(CW, n_pad - c0)
        chunks.append((c0, cw))
        c0 += cw

    KP = MAX_DEG // 2  # k-pairs

    with tile.TileContext(nc) as tc, ExitStack() as ctx:
        singles = ctx.enter_context(tc.tile_pool(name="singles", bufs=1))
        xt_p = ctx.enter_context(tc.tile_pool(name="xt", bufs=3))
        ev_p = ctx.enter_context(tc.tile_pool(name="ev", bufs=3))
        hp_p = ctx.enter_context(tc.tile_pool(name="hp", bufs=3))
        t_p = ctx.enter_context(tc.tile_pool(name="t", bufs=3))
        osb_p = ctx.enter_context(tc.tile_pool(name="osb", bufs=2))
        pm = ctx.enter_context(tc.tile_pool(name="pm", bufs=1, space="PSUM"))
        py_p = ctx.enter_context(tc.tile_pool(name="py", bufs=2, space="PSUM"))

        w1s = singles.tile([ATOM, HID], bf16)
        nc.sync.dma_start(out=w1s, in_=W1)
        w3s = singles.tile([HID, ATOM], bf16)
        nc.sync.dma_start(out=w3s, in_=W3)
        bcfs = singles.tile([HID, 1], f32)
        nc.sync.dma_start(out=bcfs, in_=BCF)
        bfcs = singles.tile([ATOM, 1], f32)
        nc.sync.dma_start(out=bfcs, in_=BFC)
        idns = singles.tile([128, 128], bf16)
        nc.sync.dma_start(out=idns, in_=IDN)

        for c0, cw in chunks:
            nj = cw // 128
            psum_y = py_p.tile([128, cw], f32)
            for kp in range(KP):
                k0 = 2 * kp
                xtw = xt_p.tile([ATOM, 2, cw], bf16)
                nc.sync.dma_start(out=xtw, in_=XT3[:, k0 : k0 + 2, c0 : c0 + cw])
                evw = ev_p.tile([HID, 2, cw], bf16)
                nc.sync.dma_start(out=evw, in_=EV3[:, k0 : k0 + 2, c0 : c0 + cw])

                p1w = pm.tile([HID, 2, cw], f32, tag="p1")
                nc.tensor.matmul(p1w[:, 0, :], w1s, xtw[:, 0, :], start=True, stop=True)
                nc.tensor.matmul(p1w[:, 1, :], w1s, xtw[:, 1, :], start=True, stop=True)

                hpw = hp_p.tile([HID, 2, cw], bf16)
                nc.vector.scalar_tensor_tensor(
                    out=hpw, in0=p1w, scalar=bcfs, in1=evw, op0=add, op1=mult
                )

                p3w = pm.tile([ATOM, 2, cw], f32, tag="p3")
                nc.tensor.matmul(p3w[:, 0, :], w3s, hpw[:, 0, :], start=True, stop=True)
                nc.tensor.matmul(p3w[:, 1, :], w3s, hpw[:, 1, :], start=True, stop=True)

                tw = t_p.tile([ATOM, 2, cw], bf16)
                nc.scalar.activation(tw, p3w, tanh, bias=bfcs)

                for kk in range(2):
                    for j in range(nj):
                        first = kp == 0 and kk == 0 and j == 0
                        last = kp == KP - 1 and kk == 1 and j == nj - 1
                        # regular matmul: psum_y += tw_slice.T @ I
                        # (transpose + neighbor-sum via fp32 PSUM accumulation)
                        nc.tensor.matmul(
                            psum_y[:, j * 128 : (j + 1) * 128],
                            tw[:, kk, j * 128 : (j + 1) * 128],
                            idns,
                            start=first,
                            stop=last,
                            skip_group_check=not (first or last),
                        )

            osb = osb_p.tile([128, cw], f32)
            nc.vector.tensor_copy(osb, psum_y)
            # osb[p, (j, o)] holds y[c0 + j*128 + p, o]
            out_view = Y[c0 : c0 + cw, :].rearrange("(j p) o -> p j o", p=128)
            nc.sync.dma_start(
                out=out_view, in_=osb.rearrange("p (j o) -> p j o", o=ATOM)
            )

    _prog_cache[n_pad] = nc
    return nc


def _split_multi_waits(nc):
    """This container's walrus codegen allows at most ONE embedded sync wait
    per instruction (setupSyncWait 'Too many sync wait commands'). Tile emits
    multi-wait sync_info; split the extras onto same-engine NoOps placed just
    before the instruction — semantically identical (engines execute their
    stream in block order)."""
    import bass_rust

    ctr = 0
    for f in nc.m.functions:
        for b in f.blocks:
            out = []
            changed = False
            for i in b.instructions:
                si = i.sync_info
                if si is not None and len(si.on_wait) > 1:
                    waits = list(si.on_wait)
                    for w in waits[:-1]:
                        nd = bass_rust.InstNoOp(name=f"wsplit-{ctr}", ins=[], outs=[])
                        ctr += 1
                        nd.engine = i.engine
                        nd.sync_info = bass_rust.SyncInfo(on_wait=[w], on_update=[])
                        out.append(nd)
                    si.on_wait = [waits[-1]]
                    changed = True
                out.append(i)
            if changed:
                b.instructions = out
    return ctr


def _pack_feature_major(shard, feat, n_pad, out_dtype, chunk=256):
    """[n, K, feat] -> [feat, K*n_pad] with k-major columns, cast to out_dtype."""
    n = shard.shape[0]
    out = np.zeros((feat, MAX_DEG, n_pad), out_dtype)
    for i in range(0, n, chunk):
        j = min(i + chunk, n)
        out[:, :, i:j] = shard[i:j].transpose(2, 1, 0).astype(out_dtype)
    return out.reshape(feat, MAX_DEG * n_pad)


def make_in_maps(src_h, he, Wcf_w, Wcf_b, Wdf_w, Wdf_b, Wfc_w, Wfc_b,
                 n_cores=N_CORES, n_per=N_PER, n_pad=N_PAD):
    import ml_dtypes

    bf16 = ml_dtypes.bfloat16
    src_h = np.asarray(src_h, np.float32)
    he = np.asarray(he, np.float32)
    Wdf_w = np.asarray(Wdf_w, np.float32)
    Wdf_b = np.asarray(Wdf_b, np.float32)
    w1 = np.ascontiguousarray(np.asarray(Wcf_w, np.float32).T).astype(bf16)  # [A,H]
    w3 = np.ascontiguousarray(np.asarray(Wfc_w, np.float32).T).astype(bf16)  # [H,O]
    bcf = np.ascontiguousarray(np.asarray(Wcf_b, np.float32).reshape(HID, 1))
    bfc = np.ascontiguousarray(np.asarray(Wfc_b, np.float32).reshape(ATOM, 1))
    idn = np.eye(128, dtype=np.float32).astype(bf16)

    in_maps = []
    for c in range(n_cores):
        xs = src_h[c * n_per : (c + 1) * n_per]
        hs = he[c * n_per : (c + 1) * n_per]
        # bond branch on host (BLAS): e_vw = he @ Wdf.T + Wdf_b
        ev = hs.reshape(-1, BOND) @ Wdf_w.T + Wdf_b          # [n*K, HID] fp32
        ev = ev.reshape(-1, MAX_DEG, HID)
        xt = _pack_feature_major(xs, ATOM, n_pad, bf16)
        evp = _pack_feature_major(ev, HID, n_pad, bf16)
        in_maps.append(dict(xt=xt, ev=evp, w1=w1, w3=w3, bcf=bcf, bfc=bfc, idn=idn))
    return in_maps


def run(inputs, trace=False):
    """Run on the 8 cores; returns (y_full [50000,128] fp32, BassKernelResults)."""
    from concourse.bass_utils import run_bass_kernel_spmd

    in_maps = make_in_maps(**inputs)
    nc = build_program()
    if not getattr(nc, "_waits_split", False):
        # walrus here allows only one embedded sync wait per instruction;
        # CoreSim can't simulate the carrier NoOps, so split only for HW.
        _split_multi_waits(nc)
        nc._waits_split = True
    res = run_bass_kernel_spmd(nc, in_maps, list(range(N_CORES)), trace=trace)
    y = np.concatenate([res.results[c]["y"][:N_PER] for c in range(N_CORES)], 0)
    return y, res


def kernel(**inputs):
    y, _ = run(inputs, trace=False)
    return y
